# revision 20
# baseline (speedup 1.0000x reference)
"""CodebookLinear TRN2 kernel, v4: W-stationary matmul + deep DMA prefetch.

Reference computation (jax):
    W = codebook[indices].reshape(-1)[:4096*4096].reshape(4096, 4096)   # [out, in]
    out = einsum('bsi,oi->bso', x, W) + bias

Distribution: 8 NeuronCores, column-parallel over out_features (each core
owns 512 output features and all 8192 tokens), no collectives.

v4 structure (the baseline was x-DMA-starved: one 1MB x tile in flight at
a time, ~21 us per tile, PE 40% idle):

  gather:   64 half-gathers (GpSimd ap_gather, 512 idx each) reconstruct
            W^T [i, o] in bf16, one k-tile = [128, 512] per pair of halves.
            Parity select on DVE as in v3. All gathers issue up-front and
            run back-to-back (~180 us total), paced by the g2 pool.
  matmul:   out^T[o, t] orientation: lhsT (stationary) = W^T k-tile slice
            [128 k, 128 o], rhs (moving) = x^T tile [128 k, 1024 t] bf16
            -> 1024-wide MMs, 1024 LDWEIGHTS total (half of v3).
            PSUM tile [128 o, 1024 t] spans 2 banks; 4 ot-tiles fill all
            8 banks per token group. k accumulates in PSUM within each of
            2 chunk passes (k 0..7 | k 8..31); pass 0 drains +bias into a
            bf16 acc, pass 1 adds acc and stores f32 out^T.
  x feed:   per (k-tile, token-group) tiles [128, 1024] f32 DMA'd on the
            sync queue with a 10-deep pool (4KB packets, ~5MB in flight),
            cast to bf16 on the Scalar engine. Out tiles DMA on the Pool
            queue (idle after gathers) so they never block x loads.

Host side only shards/reshapes: x is passed transposed and row-permuted,
indices are int16 pre-permuted into the wrapped per-group interleaved
layout (pure permutation), bias is sliced per-partition. The kernel
returns out^T [512, 8192] per core; the host transposes into
[4, 2048, 4096].

Index/partition math (per core, o local in [0, O_LOC)):
  Within k-tile it, SBUF partition p holds contraction row
      i = 128*it + sigma(p),  sigma(p) = 8*(2*(p>>4) + (p&1)) + ((p>>1)&7)
  so  j(i) = 16*it + 2*g + h,  k(i) = (p>>1)&7,  g = p>>4,  h = p&1.
  group g's list for k-tile it:  L[n = 2*o + h] = idx[o, 16*it + 2*g + h]
  wrapped storage:               idxw[16*g + q, it, f] = L[16*f + q]
  gather:  g2[p, n] = data[p, L[g(p)][n]] = cb[idx[o(n), j], k(p)]
  select:  W^T[p, o] = g2[p, 2*o + (p&1)]
  Half-gathers split n in [0,512) | [512,1024): idx columns f in [0,32)
  and [32,64) of the same wrapped layout -> o in [0,256) | [256,512).
"""

import sys

for _p in ("/opt/trn_rl_repo",):
    if _p not in sys.path:
        sys.path.insert(0, _p)

import numpy as np

import concourse.bacc as bacc
import concourse.mybir as mybir
import concourse.tile as tile
from concourse.bass_utils import run_bass_kernel_spmd

# Problem constants
OUT_F = 4096
IN_F = 4096
KCB = 4096          # codebook entries
BS = 8              # block size
JB = IN_F // BS     # 512 blocks per W row
B, S = 4, 2048
T = B * S           # 8192 tokens

# Shard grid: column-parallel over out_features
S_O = 8
O_LOC = OUT_F // S_O   # 512
T_LOC = T              # all tokens on every core

P = 128
NIT = IN_F // P        # 32 k-tiles
NOT = O_LOC // P       # 4 out tiles
FW = 2 * O_LOC // 16   # 64 wrapped index columns per k-tile
FH = FW // 2           # 32 columns per half-gather

TG = 1024              # tokens per x tile / moving-operand width
NTG = T_LOC // TG      # 8 token groups

# k-chunk passes (sum = NIT): the whole kernel is gather-paced, so chunks
# are small and matmul passes track gather progress k-outer.
CHUNKS = [4, 4, 4, 4, 4, 4, 4, 2, 2]
NCH = len(CHUNKS)
CH0 = [sum(CHUNKS[:i]) for i in range(NCH)]
GLOOK = 6              # gather emission lookahead (g2 pool has 8 bufs)
SEL_TGS = (2, 3, 4, 5)  # pass positions for next-chunk selects (2-tg delay)

# partition -> within-tile contraction row
_p_ar = np.arange(P)
SIGMA = (8 * (2 * (_p_ar >> 4) + (_p_ar & 1)) + ((_p_ar >> 1) & 7)).astype(np.int64)

_nc_cache = None
last_result = None     # BassKernelResults of the most recent run (for test.py)


def build_nc():
    nc = bacc.Bacc("TRN2", target_bir_lowering=False, debug=False)
    xT = nc.dram_tensor("xT", [IN_F, T_LOC], mybir.dt.float32, kind="ExternalInput")
    idxw = nc.dram_tensor("idxw", [P, NIT * FW], mybir.dt.int16, kind="ExternalInput")
    cbt16 = nc.dram_tensor("cbt16", [16, KCB], mybir.dt.float32,
                           kind="ExternalInput")
    bias = nc.dram_tensor("bias", [P, NOT], mybir.dt.float32, kind="ExternalInput")
    mask = nc.dram_tensor("mask", [P, 1], mybir.dt.uint8, kind="ExternalInput")
    out = nc.dram_tensor("out", [O_LOC, T_LOC], mybir.dt.float32,
                         kind="ExternalOutput")

    with tile.TileContext(nc) as tc:
        with (
            tc.tile_pool(name="const", bufs=1) as constp,
            tc.tile_pool(name="wt", bufs=1) as wtp,
            tc.tile_pool(name="acc", bufs=1) as accp,
            tc.tile_pool(name="g2p", bufs=8) as g2p,
            tc.tile_pool(name="xfp", bufs=10) as xfp,
            tc.tile_pool(name="xbp", bufs=4) as xbp,
            tc.tile_pool(name="outp", bufs=2) as outp,
            tc.tile_pool(name="psmm", bufs=8, space="PSUM") as psmm,
        ):
            # gather inputs first: the first ap_gather is the critical path
            idxt = constp.tile([P, NIT * FW], mybir.dt.int16)
            nc.sync.dma_start(out=idxt[:], in_=idxw[:, :])
            # cb^T duplicated pairwise, prepared on host
            data = constp.tile([P, KCB], mybir.dt.float32)
            for g in range(8):
                nc.sync.dma_start(
                    out=data[16 * g : 16 * (g + 1), :], in_=cbt16[:, :]
                )
            bias_t = constp.tile([P, NOT], mybir.dt.float32)
            nc.sync.dma_start(out=bias_t[:], in_=bias[:, :])
            mask_t = constp.tile([P, 1], mybir.dt.uint8)
            nc.sync.dma_start(out=mask_t[:], in_=mask[:, :])

            # W^T resident, bf16, one tile per k-tile: [sigma-row, o]
            WT = [
                wtp.tile([P, O_LOC], mybir.dt.bfloat16, name=f"WT{it}")
                for it in range(NIT)
            ]
            # out^T accumulator, bf16, one tile per (token group, out tile)
            acc = [
                accp.tile([P, TG], mybir.dt.bfloat16, name=f"acc{i}")
                for i in range(NTG * NOT)
            ]

            mask_bc = mask_t[:, 0:1].to_broadcast([P, O_LOC])
            xTr = xT[:, :].rearrange("(it p) t -> p it t", p=P)  # [128, NIT, T]

            g2_tiles = {}

            def gather_full(it):
                """ap_gather for one whole k-tile (Pool queue)."""
                g2 = g2p.tile([P, 2 * O_LOC], mybir.dt.float32, name="g2")
                g2_tiles[it] = g2
                nc.gpsimd.ap_gather(
                    out_ap=g2[:, :],
                    in_ap=data[:, :],
                    idxs_ap=idxt[:, it * FW : (it + 1) * FW],
                    channels=P,
                    num_elems=KCB,
                    d=1,
                    num_idxs=2 * O_LOC,
                )

            def select_full(it):
                """parity select -> WT[it] (DVE)."""
                g2 = g2_tiles.pop(it)
                g2_s = g2[:, :].rearrange("p (o s) -> p o s", s=2)
                dst = WT[it][:, :]
                nc.vector.tensor_copy(out=dst, in_=g2_s[:, :, 0])
                nc.vector.copy_predicated(out=dst, mask=mask_bc, data=g2_s[:, :, 1])

            sel_queue = []

            def mm_pass(c):
                """Matmul pass for k-tile chunk c over all token groups.

                Per (tg): 4 PSUM tiles [128 o, 1024 t] (2 banks each, all 8
                banks) accumulate over the chunk's k-tiles; lhsT = W^T
                slices (stationary), rhs = x^T bf16 [128, 1024] (moving).
                Drains on DVE; x casts on Scalar; chunk c+1's selects are
                emitted interleaved after each tg's drains.
                """
                first, last = (c == 0), (c == NCH - 1)
                ch = CHUNKS[c]
                k0 = CH0[c]
                for tg in range(NTG):
                    xbs = {}
                    outts = {}
                    # two half-groups of 4 PSUM banks each, so one half's
                    # drains overlap the other half's matmuls
                    for hf in range(2):
                        hs = slice(hf * (TG // 2), (hf + 1) * (TG // 2))
                        ps = [
                            psmm.tile([P, TG // 2], mybir.dt.float32, name="ps")
                            for _i in range(NOT)
                        ]
                        for itl in range(ch):
                            it = k0 + itl
                            if hf == 0:
                                xf = xfp.tile([P, TG], mybir.dt.float32, name="xf")
                                nc.sync.dma_start(
                                    out=xf[:, :],
                                    in_=xTr[:, it, tg * TG : (tg + 1) * TG],
                                )
                                xb = xbp.tile([P, TG], mybir.dt.bfloat16, name="xb")
                                nc.scalar.copy(out=xb[:, :], in_=xf[:, :])
                                xbs[it] = xb
                            for ot in range(NOT):
                                nc.tensor.matmul(
                                    out=ps[ot][:],
                                    lhsT=WT[it][:, ot * P : (ot + 1) * P],
                                    rhs=xbs[it][:, hs],
                                    start=(itl == 0),
                                    stop=(itl == ch - 1),
                                )
                        for ot in range(NOT):
                            a = acc[tg * NOT + ot]
                            if first:
                                nc.vector.tensor_tensor(
                                    out=a[:, hs], in0=ps[ot][:],
                                    in1=bias_t[:, ot : ot + 1].to_broadcast(
                                        [P, TG // 2]
                                    ),
                                    op=mybir.AluOpType.add,
                                )
                            elif not last:
                                nc.vector.tensor_tensor(
                                    out=a[:, hs], in0=ps[ot][:],
                                    in1=a[:, hs],
                                    op=mybir.AluOpType.add,
                                )
                            else:
                                if hf == 0:
                                    outts[ot] = outp.tile(
                                        [P, TG], mybir.dt.float32, name="outt"
                                    )
                                nc.vector.tensor_tensor(
                                    out=outts[ot][:, hs], in0=ps[ot][:],
                                    in1=a[:, hs],
                                    op=mybir.AluOpType.add,
                                )
                                if hf == 1:
                                    nc.gpsimd.dma_start(
                                        out=out[ot * P : (ot + 1) * P,
                                                tg * TG : (tg + 1) * TG],
                                        in_=outts[ot][:],
                                    )
                    # interleave next chunk's selects so the DVE never
                    # blocks a PSUM drain behind a gather wait: each select
                    # lands in the DVE stream ~2 tgs AFTER its gather
                    # completes (the DVE runs ~1.5 tgs ahead of the gather
                    # engine; a queued select that waits on its gather
                    # would stall the drains behind it)
                    if tg in SEL_TGS and sel_queue:
                        emit_select(sel_queue.pop(0))

            # emission: software-pipelined gather/select. Emitting all
            # gathers up-front collapses the g2 pool into a serial WAR
            # chain (slot assignment is emission-order), so each select
            # emission releases one more gather, keeping the in-flight
            # window at GLOOK tiles.
            state = {"g": 0}

            def emit_gather():
                if state["g"] < NIT:
                    gather_full(state["g"])
                    state["g"] += 1

            def emit_select(n):
                select_full(n)
                emit_gather()

            for _ in range(GLOOK):
                emit_gather()
            for n in range(CHUNKS[0]):
                emit_select(n)
            for c in range(NCH):
                if c + 1 < NCH:
                    sel_queue.extend(
                        range(CH0[c + 1], CH0[c + 1] + CHUNKS[c + 1])
                    )
                mm_pass(c)
                while sel_queue:
                    emit_select(sel_queue.pop(0))

    nc.compile()
    return nc


def _get_nc():
    global _nc_cache
    if _nc_cache is None:
        _nc_cache = build_nc()
    return _nc_cache


def _wrap_indices(idx_local):
    """[O_LOC, JB] int -> wrapped interleaved int16 [P, NIT*FW]."""
    arr = idx_local.reshape(O_LOC, NIT, 8, 2)        # [o, it, g, h]
    L = arr.transpose(2, 1, 0, 3).reshape(8, NIT, 2 * O_LOC)   # [g, it, n=2o+h]
    Lw = L.reshape(8, NIT, FW, 16)                   # [g, it, f, q]
    idxw = Lw.transpose(0, 3, 1, 2).reshape(P, NIT * FW)
    return np.ascontiguousarray(idxw.astype(np.int16))


def make_in_maps(x, codebook, indices, bias):
    x = np.asarray(x, dtype=np.float32).reshape(T, IN_F)
    xT_full = np.ascontiguousarray(x.T)  # [IN_F, T]
    # permute contraction rows within each 128-tile to match the W^T layout
    xT_perm = np.ascontiguousarray(
        xT_full.reshape(NIT, P, T)[:, SIGMA, :].reshape(IN_F, T)
    )
    idx2d = np.asarray(indices).astype(np.int64).reshape(OUT_F, JB)
    cbT_host = np.asarray(codebook, dtype=np.float32).T      # [8, 4096]
    cbt16_host = np.ascontiguousarray(
        np.repeat(cbT_host, 2, axis=0)                        # [16, 4096], rows 2k,2k+1 = cbT[k]
    )
    b = np.asarray(bias, dtype=np.float32)
    mask_np = (np.arange(P) % 2).astype(np.uint8).reshape(P, 1)

    in_maps = []
    for c in range(8):
        # bias per partition: bias_t[p, ot] = bias[c*O_LOC + ot*128 + p]
        bl = b[c * O_LOC : (c + 1) * O_LOC].reshape(NOT, P).T
        in_maps.append(
            {
                "xT": xT_perm,
                "idxw": _wrap_indices(idx2d[c * O_LOC : (c + 1) * O_LOC]),
                "cbt16": cbt16_host,
                "bias": np.ascontiguousarray(bl),
                "mask": mask_np,
            }
        )
    return in_maps


def assemble(outs):
    full = np.empty((T, OUT_F), dtype=np.float32)
    for c in range(8):
        full[:, c * O_LOC : (c + 1) * O_LOC] = outs[c]["out"].T
    return full.reshape(B, S, OUT_F)


def kernel(x, codebook, indices, bias):
    global last_result
    nc = _get_nc()
    in_maps = make_in_maps(x, codebook, indices, bias)
    last_result = run_bass_kernel_spmd(nc, in_maps, core_ids=list(range(8)))
    return assemble(last_result.results)
